# revision 5
# baseline (speedup 1.0000x reference)
"""Sparse factorization-machine forward on 8 Trainium2 NeuronCores.

Model: y = sigmoid(delta * (w0 + <x, w_bias> + u.t + t.Sb + 0.5(|Sb|^2 - sum_b||b_i||^2) + u.Sb))
where x is a sparse one/multi-hot transaction over [user(1M) | target(500K) | basket(500K)],
u = x_u @ u_V, t = x_t @ t_V, Sb = x_b @ b_V.

Strategy (embedding/model parallel, row sharding):
 - each core owns 1/8 of the rows of u_V/t_V/b_V/w_bias and the matching x slices
 - on device: scan the x slices for nonzeros (masked-iota keys + per-partition top-8),
   build int16 gather lists, dma_gather only the needed embedding rows from HBM,
   weighted-sum them, and reduce partials across partitions with PE matmuls
 - bias term <x, w_bias> is computed densely on-device (it is tiny)
 - partials (u/t/Sb/q vectors + bias scalar) are all-reduced across the 8 cores and
   the scalar FM combination + sigmoid computed on-device (collective mode), or
   returned per-core and combined on the host (a pure unshard-reduce + epilogue)
"""
import os
import sys

sys.path.insert(0, '/opt/trn_rl_repo')

import numpy as np

N_USERS = 1_000_000
N_ITEMS = 500_000
K = 64
NCORES = 8
GAMMA = (1.0, 1.0, 1.0, 1.0)

F = 256                 # free elems per partition per sub-range
SUB = 128 * F           # 32768 rows per sub-range (int16 gather index limit)
U_SH = N_USERS // NCORES     # 125000 rows/core
T_SH = N_ITEMS // NCORES     # 62500
B_SH = N_ITEMS // NCORES     # 62500
RU, RT, RB = 4, 2, 2    # sub-ranges per core per table
K_UT = 1                # gather slots per partition (one-hot ranges)
K_B = 4                 # gather slots per partition (basket range)

COLLECTIVE = os.environ.get("BFM_COLLECTIVE", "1") == "1"

_cache = {}


def _split_multi_waits(nc):
    """This walrus build accepts at most one sem-wait per instruction; hoist
    extras into standalone event-semaphore waits on the same engine."""
    from concourse import mybir
    n = 0
    for bb in nc.main_func.blocks:
        insts = bb.instructions
        out = []
        for ins in insts:
            si = ins.sync_info
            waits = list(si.on_wait) if si and si.on_wait else []
            if len(waits) > 1:
                for w in waits[1:]:
                    ev = mybir.InstEventSemaphore(name=f"{ins.name}-hw{n}", ins=[], outs=[])
                    ev.engine = ins.engine
                    ev.sync_info = mybir.SyncInfo(on_wait=[w], on_update=[])
                    out.append(ev)
                    n += 1
                si.on_wait = waits[:1]
            out.append(ins)
        if len(out) != len(insts):
            insts[:] = out
    return n


def _build(collective: bool):
    from concourse import bacc, mybir, tile, library_config

    fp32 = mybir.dt.float32
    i16 = mybir.dt.int16
    Alu = mybir.AluOpType

    nc = bacc.Bacc("TRN2", num_devices=NCORES)

    xu_d = nc.dram_tensor("xu", [U_SH], fp32, kind="ExternalInput")
    xt_d = nc.dram_tensor("xt", [T_SH], fp32, kind="ExternalInput")
    xb_d = nc.dram_tensor("xb", [B_SH], fp32, kind="ExternalInput")
    wu_d = nc.dram_tensor("wu", [U_SH], fp32, kind="ExternalInput")
    wt_d = nc.dram_tensor("wt", [T_SH], fp32, kind="ExternalInput")
    wb_d = nc.dram_tensor("wb", [B_SH], fp32, kind="ExternalInput")
    uV_d = nc.dram_tensor("uV", [U_SH, K], fp32, kind="ExternalInput")
    tV_d = nc.dram_tensor("tV", [T_SH, K], fp32, kind="ExternalInput")
    bV_d = nc.dram_tensor("bV", [B_SH, K], fp32, kind="ExternalInput")
    i1_d = nc.dram_tensor("iota1", [128, F], fp32, kind="ExternalInput")
    part_d = nc.dram_tensor("part", [64, 6], fp32, kind="ExternalOutput")
    if collective:
        co_d = nc.dram_tensor("coef5", [8, 1], fp32, kind="ExternalInput")
        w0_d = nc.dram_tensor("w0d", [1], fp32, kind="ExternalInput")
        de_d = nc.dram_tensor("delta", [1], fp32, kind="ExternalInput")
        out_d = nc.dram_tensor("out", [1, 1], fp32, kind="ExternalOutput")

    # sub-range descriptors: (tag, r, x_tile, w_tile, col_block, slot_off, k, tbl, row0, nrows)
    ranges = [
        ("u", RU, U_SH, xu_d, wu_d, uV_d, K_UT),
        ("t", RT, T_SH, xt_d, wt_d, tV_d, K_UT),
        ("b", RB, B_SH, xb_d, wb_d, bV_d, K_B),
    ]

    with tile.TileContext(nc) as tc:
        with (
            tc.tile_pool(name="sbuf", bufs=1) as pool,
            tc.tile_pool(name="psum", bufs=1, space="PSUM") as psum,
            tc.tile_pool(name="dram", bufs=1, space="DRAM") as dram,
        ):
            nc.gpsimd.load_library(library_config.mlp)

            I1 = pool.tile([128, F], fp32)
            nc.sync.dma_start(I1[:], i1_d[:])

            # ---- load x and w_bias slices, padded into (128, R*F) tiles ----
            xw_tiles = {}
            for tag, R, SH, x_d, w_d, _tbl, _k in ranges:
                for nm, src in (("x", x_d), ("w", w_d)):
                    t = pool.tile([128, R * F], fp32, tag=f"{nm}{tag}")
                    full_rows = SH // F            # full 256-elem rows
                    tail = SH - full_rows * F
                    r_full = full_rows // 128      # whole sub-ranges fully covered
                    p_rem = full_rows - r_full * 128
                    # zero the ragged sub-range's block, then overwrite with data
                    if r_full < R:
                        nc.vector.memset(t[:, r_full * F:(r_full + 1) * F], 0.0)
                    if r_full > 0:
                        nc.sync.dma_start(
                            t[:, 0:r_full * F].rearrange("p (r f) -> p r f", f=F),
                            src[0:r_full * SUB].rearrange("(r p f) -> p r f", p=128, f=F),
                        )
                    if p_rem > 0:
                        nc.sync.dma_start(
                            t[:p_rem, r_full * F:(r_full + 1) * F],
                            src[r_full * SUB: r_full * SUB + p_rem * F]
                            .rearrange("(p f) -> p f", f=F),
                        )
                    if tail > 0:
                        nc.sync.dma_start(
                            t[p_rem:p_rem + 1, r_full * F: r_full * F + tail],
                            src[r_full * SUB + p_rem * F: SH].rearrange("(a f) -> a f", a=1),
                        )
                    xw_tiles[f"{nm}{tag}"] = t

            # ---- scan: keys = x * (local_idx+1); top-8 per partition ----
            # sub-range order: u0..u3, t0, t1, b0, b1
            T8S = pool.tile([128, 8 * 8], fp32)
            W8S = pool.tile([128, 14], fp32)
            IDXS = pool.tile([128, 14], i16)
            KEY = pool.tile([128, F], fp32)
            ridx = 0
            soff = 0
            slot_of = {}
            for tag, R, SH, _x, _w, _tbl, k in ranges:
                xt_tile = xw_tiles[f"x{tag}"]
                for r in range(R):
                    nc.vector.tensor_tensor(
                        KEY[:], xt_tile[:, r * F:(r + 1) * F], I1[:], Alu.mult)
                    nc.vector.max(T8S[:, 8 * ridx:8 * ridx + 8], KEY[:])
                    nc.vector.tensor_scalar(
                        W8S[:, soff:soff + k], T8S[:, 8 * ridx:8 * ridx + k],
                        1.0, None, Alu.is_ge)
                    nc.vector.tensor_scalar(
                        IDXS[:, soff:soff + k], T8S[:, 8 * ridx:8 * ridx + k],
                        1.0, 0.0, Alu.subtract, Alu.max)
                    slot_of[(tag, r)] = soff
                    ridx += 1
                    soff += k

            # ---- fold idx lists into 16-partition-wrapped layout via DRAM ----
            scr_ut = dram.tile([6 * 128], i16)
            scr_b = dram.tile([2 * 512], i16)
            for r in range(6):
                nc.sync.dma_start(
                    scr_ut[128 * r:128 * (r + 1)]
                    .rearrange("(q m) -> m q", q=16, m=8),
                    IDXS[:, r:r + 1])
            for r in range(2):
                nc.sync.dma_start(
                    scr_b[512 * r:512 * (r + 1)]
                    .rearrange("(q s m) -> m q s", q=16, s=4, m=8),
                    IDXS[:, 6 + 4 * r:10 + 4 * r])
            IDXW = pool.tile([128, 112], i16)
            for g in range(8):
                nc.sync.dma_start(
                    IDXW[16 * g:16 * (g + 1), 0:48]
                    .rearrange("q (r m) -> q r m", r=6),
                    scr_ut[:].rearrange("(r q m) -> q r m", r=6, q=16, m=8))
                nc.sync.dma_start(
                    IDXW[16 * g:16 * (g + 1), 48:112]
                    .rearrange("q (r j) -> q r j", r=2),
                    scr_b[:].rearrange("(r q j) -> q r j", r=2, q=16, j=32))

            # ---- gather embedding rows ----
            G = {}
            gofs = {"u": 0, "t": 32, "b": 48}
            for tag, R, SH, _x, _w, tbl, k in ranges:
                g_t = pool.tile([128, R * k, K], fp32, tag=f"G{tag}")
                for r in range(R):
                    row0 = r * SUB
                    nrows = min(SUB, SH - row0)
                    nidx = 128 * k
                    width = nidx // 16
                    off = gofs[tag] + r * width
                    nc.gpsimd.dma_gather(
                        out_ap=g_t[:, r * k:(r + 1) * k, :],
                        in_ap=tbl[row0:row0 + nrows, :],
                        idxs_ap=IDXW[:, off:off + width],
                        num_idxs=nidx, num_idxs_reg=nidx, elem_size=K,
                    )
                G[tag] = g_t

            # ---- weighted sums: ACC_tag[p, d] = sum_slots w * row ----
            ACC = {}
            for tag, R, SH, _x, _w, _tbl, k in ranges:
                g_t = G[tag]
                nslots = R * k
                tw = pool.tile([128, nslots, K], fp32, tag=f"TW{tag}")
                for r in range(R):
                    for s in range(k):
                        w_ap = W8S[:, slot_of[(tag, r)] + s: slot_of[(tag, r)] + s + 1]
                        nc.vector.tensor_scalar(
                            tw[:, r * k + s, :], g_t[:, r * k + s, :],
                            w_ap, None, Alu.mult)
                acc = pool.tile([128, K], fp32, tag=f"ACC{tag}")
                nc.vector.tensor_reduce(
                    acc[:],
                    tw[:].rearrange("p s d -> p d s"),
                    axis=mybir.AxisListType.X, op=Alu.add)
                ACC[tag] = acc

            # ---- q = sum_slots w * row^2 (basket only) ----
            gb = G["b"]
            nb = RB * K_B
            SQ = pool.tile([128, nb, K], fp32)
            nc.vector.tensor_tensor(SQ[:], gb[:], gb[:], Alu.mult)
            SQW = pool.tile([128, nb, K], fp32)
            for r in range(RB):
                for s in range(K_B):
                    w_ap = W8S[:, slot_of[("b", r)] + s: slot_of[("b", r)] + s + 1]
                    nc.vector.tensor_scalar(
                        SQW[:, r * K_B + s, :], SQ[:, r * K_B + s, :],
                        w_ap, None, Alu.mult)
            ACCQ = pool.tile([128, K], fp32)
            nc.vector.tensor_reduce(
                ACCQ[:], SQW[:].rearrange("p s d -> p d s"),
                axis=mybir.AxisListType.X, op=Alu.add)

            # ---- bias = <x, w_bias> over all three slices ----
            PROD = pool.tile([128, RU * F], fp32)
            bias_acc = pool.tile([128, 4], fp32)
            for i, (tag, R, SH, _x, _w, _tbl, _k) in enumerate(ranges):
                nc.vector.tensor_tensor(
                    PROD[:, 0:R * F], xw_tiles[f"x{tag}"][:], xw_tiles[f"w{tag}"][:],
                    Alu.mult)
                nc.vector.tensor_reduce(
                    bias_acc[:, i:i + 1], PROD[:, 0:R * F],
                    axis=mybir.AxisListType.X, op=Alu.add)
            nc.vector.tensor_tensor(
                bias_acc[:, 3:4], bias_acc[:, 0:1], bias_acc[:, 1:2], Alu.add)
            nc.vector.tensor_tensor(
                bias_acc[:, 2:3], bias_acc[:, 3:4], bias_acc[:, 2:3], Alu.add)

            # ---- cross-partition reduce via PE (data stationary, ones moving) ----
            ones = pool.tile([128, 1], fp32)
            nc.vector.memset(ones[:], 1.0)
            PART = pool.tile([64, 6], fp32)
            nc.vector.memset(PART[:], 0.0)
            for i, tag in enumerate(("u", "t", "b")):
                pp = psum.tile([K, 1], fp32, tag=f"P{tag}")
                nc.tensor.matmul(pp[:], ACC[tag][:], ones[:], start=True, stop=True)
                nc.vector.tensor_copy(PART[:, i:i + 1], pp[:])
            ppq = psum.tile([K, 1], fp32, tag="Pq")
            nc.tensor.matmul(ppq[:], ACCQ[:], ones[:], start=True, stop=True)
            nc.vector.tensor_copy(PART[:, 3:4], ppq[:])
            ppb = psum.tile([1, 1], fp32, tag="Pbias")
            nc.tensor.matmul(ppb[:], bias_acc[:, 2:3], ones[:], start=True, stop=True)
            nc.vector.tensor_copy(PART[0:1, 4:5], ppb[:])

            nc.sync.dma_start(part_d[:], PART[:])

            if collective:
                in_b = dram.tile([64 * 6], fp32)
                out_b = dram.tile([64 * 6], fp32)
                nc.sync.dma_start(in_b[:].rearrange("(p c) -> p c", c=6), PART[:])
                nc.gpsimd.collective_compute(
                    "AllReduce", Alu.add,
                    replica_groups=[list(range(NCORES))],
                    ins=[in_b.opt()], outs=[out_b.opt()],
                )
                RED = pool.tile([64, 6], fp32)
                nc.sync.dma_start(RED[:], out_b[:].rearrange("(p c) -> p c", c=6))
                # dots: [u.t, t.sb, sb.sb, u.sb, sum q]
                D = pool.tile([64, 5], fp32)
                nc.vector.tensor_tensor(D[:, 0:1], RED[:, 0:1], RED[:, 1:2], Alu.mult)
                nc.vector.tensor_tensor(D[:, 1:2], RED[:, 1:2], RED[:, 2:3], Alu.mult)
                nc.vector.tensor_tensor(D[:, 2:3], RED[:, 2:3], RED[:, 2:3], Alu.mult)
                nc.vector.tensor_tensor(D[:, 3:4], RED[:, 0:1], RED[:, 2:3], Alu.mult)
                nc.vector.tensor_copy(D[:, 4:5], RED[:, 3:4])
                ones64 = ones[:64, :]
                PD = psum.tile([5, 1], fp32)
                nc.tensor.matmul(PD[:], D[:], ones64, start=True, stop=True)
                CO = pool.tile([8, 1], fp32)
                nc.sync.dma_start(CO[:], co_d[:])
                FD = pool.tile([5, 1], fp32)
                nc.vector.tensor_tensor(FD[:], PD[:], CO[:5, :], Alu.mult)
                PY = psum.tile([1, 1], fp32)
                nc.tensor.matmul(PY[:], FD[:], ones[:5, :], start=True, stop=True)
                W0T = pool.tile([1, 2], fp32)
                nc.sync.dma_start(W0T[0, 0:1], w0_d[:])
                nc.sync.dma_start(W0T[0, 1:2], de_d[:])
                Y = pool.tile([1, 3], fp32)
                nc.vector.tensor_tensor(Y[:, 0:1], PY[:], RED[0:1, 4:5], Alu.add)
                nc.vector.tensor_tensor(Y[:, 1:2], Y[:, 0:1], W0T[:, 0:1], Alu.add)
                nc.vector.tensor_tensor(Y[:, 2:3], Y[:, 1:2], W0T[:, 1:2], Alu.mult)
                OUT = pool.tile([1, 1], fp32)
                nc.scalar.activation(
                    OUT[:], Y[:, 2:3], mybir.ActivationFunctionType.Sigmoid)
                nc.sync.dma_start(out_d[:], OUT[:])

    nc.compile()
    return nc


def _get_nc(collective: bool):
    key = ("nc", collective)
    if key not in _cache:
        nc = _build(collective)
        _split_multi_waits(nc)
        _cache[key] = nc
    return _cache[key]


def kernel(x, delta, pmi, w_0, w_bias, u_V, t_V, b_V):
    from concourse.bass_utils import run_bass_kernel_spmd

    x = np.asarray(x, dtype=np.float32)
    w_bias_f = np.asarray(w_bias, dtype=np.float32).reshape(-1)
    u_V = np.ascontiguousarray(np.asarray(u_V, dtype=np.float32))
    t_V = np.ascontiguousarray(np.asarray(t_V, dtype=np.float32))
    b_V = np.ascontiguousarray(np.asarray(b_V, dtype=np.float32))
    w0 = np.asarray(w_0, dtype=np.float32).reshape(-1)
    delta_f = np.asarray(delta, dtype=np.float32).reshape(-1)

    i1_np = (np.arange(SUB, dtype=np.float32).reshape(128, F) + 1.0)
    coef5 = np.array([[GAMMA[0]], [GAMMA[1]], [0.5 * GAMMA[2]], [GAMMA[3]],
                      [-0.5 * GAMMA[2]], [0.0], [0.0], [0.0]], dtype=np.float32)

    in_maps = []
    for c in range(NCORES):
        m = {
            "xu": x[c * U_SH:(c + 1) * U_SH],
            "xt": x[N_USERS + c * T_SH: N_USERS + (c + 1) * T_SH],
            "xb": x[N_USERS + N_ITEMS + c * B_SH: N_USERS + N_ITEMS + (c + 1) * B_SH],
            "wu": w_bias_f[c * U_SH:(c + 1) * U_SH],
            "wt": w_bias_f[N_USERS + c * T_SH: N_USERS + (c + 1) * T_SH],
            "wb": w_bias_f[N_USERS + N_ITEMS + c * B_SH: N_USERS + N_ITEMS + (c + 1) * B_SH],
            "uV": u_V[c * U_SH:(c + 1) * U_SH],
            "tV": t_V[c * T_SH:(c + 1) * T_SH],
            "bV": b_V[c * B_SH:(c + 1) * B_SH],
            "iota1": i1_np,
        }
        if COLLECTIVE:
            m["coef5"] = coef5
            m["w0d"] = w0
            m["delta"] = delta_f
        in_maps.append(m)

    nc = _get_nc(COLLECTIVE)
    trace = os.environ.get("BFM_TRACE", "0") == "1"
    kwargs = {}
    if trace:
        kwargs["trace"] = True
    res = run_bass_kernel_spmd(nc, in_maps, core_ids=list(range(NCORES)), **kwargs)
    kernel._last_results = res

    if COLLECTIVE:
        return np.asarray(res.results[0]["out"], dtype=np.float32).reshape(1, 1)

    # host epilogue: unshard (sum partial outputs) + scalar FM combination
    P = np.zeros((64, 6), dtype=np.float64)
    for c in range(NCORES):
        P += np.asarray(res.results[c]["part"], dtype=np.float64)
    u, t, sb, q = P[:, 0], P[:, 1], P[:, 2], P[:, 3]
    bias = P[0, 4]
    y = (float(w0[0]) + bias + GAMMA[0] * (u @ t) + GAMMA[1] * (t @ sb)
         + GAMMA[2] * 0.5 * ((sb @ sb) - q.sum()) + GAMMA[3] * (u @ sb))
    z = y * float(delta_f[0])
    out = 1.0 / (1.0 + np.exp(-z))
    return np.array([[out]], dtype=np.float32)


# revision 12
# speedup vs baseline: 1.1798x; 1.1798x over previous
"""Sparse factorization-machine forward on 8 Trainium2 NeuronCores.

Model: y = sigmoid(delta * (w0 + <x, w_bias> + u.t + t.Sb + 0.5(|Sb|^2 - sum_b||b_i||^2) + u.Sb))
where x is a sparse one/multi-hot transaction over [user(1M) | target(500K) | basket(500K)],
u = x_u @ u_V, t = x_t @ t_V, Sb = x_b @ b_V.

Strategy (embedding/model parallel, row sharding):
 - each core owns 1/8 of the rows of u_V/t_V/b_V/w_bias and the matching x slices
 - on device: scan the x slices for nonzeros (masked-iota keys + per-partition top-8),
   build int16 gather lists, dma_gather only the needed embedding rows from HBM,
   weighted-sum them, and reduce partials across partitions with PE matmuls
 - bias term <x, w_bias> is computed densely on-device (it is tiny)
 - partials (u/t/Sb/q vectors + bias scalar) are all-reduced across the 8 cores and
   the scalar FM combination + sigmoid computed on-device (collective mode), or
   returned per-core and combined on the host (a pure unshard-reduce + epilogue)
"""
import os
import sys

sys.path.insert(0, '/opt/trn_rl_repo')

import numpy as np

N_USERS = 1_000_000
N_ITEMS = 500_000
K = 64
NCORES = 8
GAMMA = (1.0, 1.0, 1.0, 1.0)

F = 256                 # free elems per partition per sub-range
SUB = 128 * F           # 32768 rows per sub-range (int16 gather index limit)
U_SH = N_USERS // NCORES     # 125000 rows/core
T_SH = N_ITEMS // NCORES     # 62500
B_SH = N_ITEMS // NCORES     # 62500
RU, RT, RB = 4, 2, 2    # sub-ranges per core per table
K_UT = 1                # gather slots per partition (one-hot ranges)
K_B = 3                 # gather slots per partition (basket range)

COLLECTIVE = os.environ.get("BFM_COLLECTIVE", "1") == "1"

_cache = {}


def _split_multi_waits(nc):
    """This walrus build accepts at most one sem-wait per instruction; hoist
    extras into standalone event-semaphore waits on the same engine."""
    from concourse import mybir
    n = 0
    for bb in nc.main_func.blocks:
        insts = bb.instructions
        out = []
        for ins in insts:
            si = ins.sync_info
            waits = list(si.on_wait) if si and si.on_wait else []
            if len(waits) > 1:
                for w in waits[1:]:
                    ev = mybir.InstEventSemaphore(name=f"{ins.name}-hw{n}", ins=[], outs=[])
                    ev.engine = ins.engine
                    ev.sync_info = mybir.SyncInfo(on_wait=[w], on_update=[])
                    out.append(ev)
                    n += 1
                si.on_wait = waits[:1]
            out.append(ins)
        if len(out) != len(insts):
            insts[:] = out
    return n


def _build(collective: bool):
    from concourse import bacc, mybir, tile, library_config

    fp32 = mybir.dt.float32
    i16 = mybir.dt.int16
    Alu = mybir.AluOpType

    nc = bacc.Bacc("TRN2", num_devices=NCORES, num_swdge_queues=4)

    XAW = 8 * F            # 2048 cols: u blocks 0-3, t 4-5, b 6-7
    NS = RU * K_UT + RT * K_UT + RB * K_B   # 12 gather slots

    xall_d = nc.dram_tensor("xall", [8 * SUB], fp32, kind="ExternalInput")
    wall_d = nc.dram_tensor("wall", [8 * SUB], fp32, kind="ExternalInput")
    uV_d = nc.dram_tensor("uV", [U_SH, K], fp32, kind="ExternalInput")
    tV_d = nc.dram_tensor("tV", [T_SH, K], fp32, kind="ExternalInput")
    bV_d = nc.dram_tensor("bV", [B_SH, K], fp32, kind="ExternalInput")
    i1_d = nc.dram_tensor("iota1", [128, XAW], fp32, kind="ExternalInput")
    part_d = nc.dram_tensor("part", [64, 6], fp32, kind="ExternalOutput")
    if collective:
        co_d = nc.dram_tensor("coef5", [8, 1], fp32, kind="ExternalInput")
        w0_d = nc.dram_tensor("w0d", [1], fp32, kind="ExternalInput")
        de_d = nc.dram_tensor("delta", [1], fp32, kind="ExternalInput")
        out_d = nc.dram_tensor("out", [1, 1], fp32, kind="ExternalOutput")

    # (tag, num sub-ranges, shard rows, table, slots per partition, slot offset, block offset)
    ranges = [
        ("u", RU, U_SH, uV_d, K_UT, 0, 0),
        ("t", RT, T_SH, tV_d, K_UT, RU * K_UT, RU),
        ("b", RB, B_SH, bV_d, K_B, RU * K_UT + RT * K_UT, RU + RT),
    ]

    with tile.TileContext(nc) as tc:
        with (
            tc.tile_pool(name="sbuf", bufs=1) as pool,
            tc.tile_pool(name="psum", bufs=1, space="PSUM") as psum,
            tc.tile_pool(name="dram", bufs=1, space="DRAM") as dram,
        ):
            nc.gpsimd.load_library(library_config.mlp)

            XA = pool.tile([128, XAW], fp32)
            IA = pool.tile([128, XAW], fp32)
            nc.sync.dma_start(
                XA[:].rearrange("p (r f) -> p r f", f=F),
                xall_d[:].rearrange("(r p f) -> p r f", p=128, f=F))
            nc.sync.dma_start(IA[:], i1_d[:])

            # ---- scan: keys = x * (local_idx+1), top-8 per partition ----
            KEYA = pool.tile([128, XAW], fp32)
            nc.vector.tensor_tensor(KEYA[:], XA[:], IA[:], Alu.mult)
            T8S = pool.tile([128, 64], fp32)
            for r in range(8):
                nc.vector.max(T8S[:, 8 * r:8 * r + 8], KEYA[:, r * F:(r + 1) * F])

            # weights / int16 indices for the 12 gather slots
            W8S = pool.tile([128, NS], fp32)
            IDXALL = pool.tile([128, NS], i16)
            ut_keys = T8S[:, 0:48].rearrange("p (r e) -> p r e", e=8)[:, :, 0:K_UT]
            b_keys = T8S[:, 48:64].rearrange("p (r e) -> p r e", e=8)[:, :, 0:K_B]
            nc.vector.tensor_scalar(
                W8S[:, 0:6].rearrange("p (r e) -> p r e", e=K_UT),
                ut_keys, 1.0, None, Alu.is_ge)
            nc.vector.tensor_scalar(
                W8S[:, 6:NS].rearrange("p (r e) -> p r e", e=K_B),
                b_keys, 1.0, None, Alu.is_ge)
            nc.vector.tensor_scalar(
                IDXALL[:, 0:6].rearrange("p (r e) -> p r e", e=K_UT),
                ut_keys, 1.0, 0.0, Alu.subtract, Alu.max)
            nc.vector.tensor_scalar(
                IDXALL[:, 6:NS].rearrange("p (r e) -> p r e", e=K_B),
                b_keys, 1.0, 0.0, Alu.subtract, Alu.max)

            # ---- fold idx lists into 16-partition-wrapped layout via DRAM ----
            IW = NS * 8                    # 96 idx cols per partition row
            scr = dram.tile([16 * IW], i16)
            scr_v = scr[:].rearrange("(q w) -> w q", q=16, w=IW)
            eng_cycle = [nc.sync, nc.scalar]
            for j in range(NS):
                eng_cycle[j % 2].dma_start(
                    scr_v[8 * j:8 * j + 8], IDXALL[:, j:j + 1])
            IDXW = pool.tile([128, IW], i16)
            for g in range(8):
                eng_cycle[g % 2].dma_start(
                    IDXW[16 * g:16 * (g + 1), :],
                    scr[:].rearrange("(q w) -> q w", w=IW))

            # ---- gather embedding rows (SWDGE, spread across queues) ----
            GA = pool.tile([128, NS, K], fp32)
            qn = 0
            for tag, R, SH, tbl, k, soff, _blk in ranges:
                for r in range(R):
                    row0 = r * SUB
                    nrows = min(SUB, SH - row0)
                    for sslot in range(k):
                        j = soff + r * k + sslot
                        nc.gpsimd.dma_gather(
                            out_ap=GA[:, j:j + 1, :],
                            in_ap=tbl[row0:row0 + nrows, :],
                            idxs_ap=IDXW[:, 8 * j:8 * j + 8],
                            num_idxs=128, num_idxs_reg=128, elem_size=K,
                            queue_num=qn % 4,
                        )
                        qn += 1

            # ---- weighted sums ----
            WM = pool.tile([128, NS, K], fp32)
            wb_ap = W8S[:].rearrange("p (s one) -> p s one", one=1)
            wb_ap.ap[-1] = [0, K]
            nc.vector.tensor_tensor(WM[:], GA[:], wb_ap, Alu.mult)
            ACC = {}
            for tag, cols in (("u", (0, RU)), ("t", (RU, RU + RT)),
                              ("b", (6, NS))):
                a = pool.tile([128, K], fp32, tag=f"ACC{tag}")
                lo, hi = cols
                nc.vector.tensor_reduce(
                    a[:],
                    WM[:, lo:hi, :].rearrange("p s d -> p d s"),
                    axis=mybir.AxisListType.X, op=Alu.add)
                ACC[tag] = a

            # q = sum w * row^2 over basket slots
            nbs = RB * K_B
            SQ = pool.tile([128, nbs, K], fp32)
            nc.vector.tensor_tensor(SQ[:], GA[:, 6:NS, :], GA[:, 6:NS, :], Alu.mult)
            SQW = pool.tile([128, nbs, K], fp32)
            wqb_ap = W8S[:, 6:NS].rearrange("p (s one) -> p s one", one=1)
            wqb_ap.ap[-1] = [0, K]
            nc.vector.tensor_tensor(SQW[:], SQ[:], wqb_ap, Alu.mult)
            ACCQ = pool.tile([128, K], fp32)
            nc.vector.tensor_reduce(
                ACCQ[:], SQW[:].rearrange("p s d -> p d s"),
                axis=mybir.AxisListType.X, op=Alu.add)

            # ---- bias = <x, w_bias> ----
            WA = pool.tile([128, XAW], fp32)
            nc.scalar.dma_start(
                WA[:].rearrange("p (r f) -> p r f", f=F),
                wall_d[:].rearrange("(r p f) -> p r f", p=128, f=F))
            BP = pool.tile([128, XAW], fp32)
            nc.vector.tensor_tensor(BP[:], XA[:], WA[:], Alu.mult)
            bias_acc = pool.tile([128, 1], fp32)
            nc.vector.tensor_reduce(
                bias_acc[:], BP[:], axis=mybir.AxisListType.X, op=Alu.add)

            # ---- cross-partition reduce via PE (data stationary, ones moving) ----
            ones = pool.tile([128, 1], fp32)
            nc.vector.memset(ones[:], 1.0)
            PART = pool.tile([64, 6], fp32)
            nc.vector.memset(PART[:], 0.0)
            for i, tag in enumerate(("u", "t", "b")):
                pp = psum.tile([K, 1], fp32, tag=f"P{tag}")
                nc.tensor.matmul(pp[:], ACC[tag][:], ones[:], start=True, stop=True)
                nc.vector.tensor_copy(PART[:, i:i + 1], pp[:])
            ppq = psum.tile([K, 1], fp32, tag="Pq")
            nc.tensor.matmul(ppq[:], ACCQ[:], ones[:], start=True, stop=True)
            nc.vector.tensor_copy(PART[:, 3:4], ppq[:])
            ppb = psum.tile([1, 1], fp32, tag="Pbias")
            nc.tensor.matmul(ppb[:], bias_acc[:], ones[:], start=True, stop=True)
            nc.vector.tensor_copy(PART[0:1, 4:5], ppb[:])

            nc.sync.dma_start(part_d[:], PART[:])

            if collective:
                in_b = dram.tile([64 * 6], fp32)
                out_b = dram.tile([64 * 6], fp32)
                nc.sync.dma_start(in_b[:].rearrange("(p c) -> p c", c=6), PART[:])
                nc.gpsimd.collective_compute(
                    "AllReduce", Alu.add,
                    replica_groups=[list(range(NCORES))],
                    ins=[in_b.opt()], outs=[out_b.opt()],
                )
                RED = pool.tile([64, 6], fp32)
                nc.sync.dma_start(RED[:], out_b[:].rearrange("(p c) -> p c", c=6))
                D = pool.tile([64, 5], fp32)
                nc.vector.tensor_tensor(D[:, 0:1], RED[:, 0:1], RED[:, 1:2], Alu.mult)
                nc.vector.tensor_tensor(D[:, 1:2], RED[:, 1:2], RED[:, 2:3], Alu.mult)
                nc.vector.tensor_tensor(D[:, 2:3], RED[:, 2:3], RED[:, 2:3], Alu.mult)
                nc.vector.tensor_tensor(D[:, 3:4], RED[:, 0:1], RED[:, 2:3], Alu.mult)
                nc.vector.tensor_copy(D[:, 4:5], RED[:, 3:4])
                PD = psum.tile([5, 1], fp32)
                nc.tensor.matmul(PD[:], D[:], ones[:64, :], start=True, stop=True)
                CO = pool.tile([8, 1], fp32)
                nc.sync.dma_start(CO[:], co_d[:])
                FD = pool.tile([5, 1], fp32)
                nc.vector.tensor_tensor(FD[:], PD[:], CO[:5, :], Alu.mult)
                PY = psum.tile([1, 1], fp32)
                nc.tensor.matmul(PY[:], FD[:], ones[:5, :], start=True, stop=True)
                W0T = pool.tile([1, 2], fp32)
                nc.sync.dma_start(W0T[0:1, 0:1], w0_d[:].rearrange("(a b) -> a b", a=1))
                nc.sync.dma_start(W0T[0:1, 1:2], de_d[:].rearrange("(a b) -> a b", a=1))
                Y = pool.tile([1, 3], fp32)
                nc.vector.tensor_tensor(Y[:, 0:1], PY[:], RED[0:1, 4:5], Alu.add)
                nc.vector.tensor_tensor(Y[:, 1:2], Y[:, 0:1], W0T[:, 0:1], Alu.add)
                nc.vector.tensor_tensor(Y[:, 2:3], Y[:, 1:2], W0T[:, 1:2], Alu.mult)
                OUT = pool.tile([1, 1], fp32)
                nc.scalar.activation(
                    OUT[:], Y[:, 2:3], mybir.ActivationFunctionType.Sigmoid)
                nc.sync.dma_start(out_d[:], OUT[:])

    nc.compile()
    return nc


def _get_nc(collective: bool):
    key = ("nc", collective)
    if key not in _cache:
        nc = _build(collective)
        _split_multi_waits(nc)
        _cache[key] = nc
    return _cache[key]


def kernel(x, delta, pmi, w_0, w_bias, u_V, t_V, b_V):
    from concourse.bass_utils import run_bass_kernel_spmd

    x = np.asarray(x, dtype=np.float32)
    w_bias_f = np.asarray(w_bias, dtype=np.float32).reshape(-1)
    u_V = np.ascontiguousarray(np.asarray(u_V, dtype=np.float32))
    t_V = np.ascontiguousarray(np.asarray(t_V, dtype=np.float32))
    b_V = np.ascontiguousarray(np.asarray(b_V, dtype=np.float32))
    w0 = np.asarray(w_0, dtype=np.float32).reshape(-1)
    delta_f = np.asarray(delta, dtype=np.float32).reshape(-1)

    PADR = 8 * SUB
    xall = np.zeros(PADR, dtype=np.float32)
    wall = np.zeros(PADR, dtype=np.float32)
    i1_np = np.tile(np.arange(SUB, dtype=np.float32).reshape(128, F) + 1.0, (1, 8))
    coef5 = np.array([[GAMMA[0]], [GAMMA[1]], [0.5 * GAMMA[2]], [GAMMA[3]],
                      [-0.5 * GAMMA[2]], [0.0], [0.0], [0.0]], dtype=np.float32)

    in_maps = []
    for c in range(NCORES):
        xa = np.zeros(PADR, dtype=np.float32)
        wa = np.zeros(PADR, dtype=np.float32)
        xa[0:U_SH] = x[c * U_SH:(c + 1) * U_SH]
        xa[RU * SUB:RU * SUB + T_SH] = x[N_USERS + c * T_SH: N_USERS + (c + 1) * T_SH]
        xa[(RU + RT) * SUB:(RU + RT) * SUB + B_SH] = \
            x[N_USERS + N_ITEMS + c * B_SH: N_USERS + N_ITEMS + (c + 1) * B_SH]
        wa[0:U_SH] = w_bias_f[c * U_SH:(c + 1) * U_SH]
        wa[RU * SUB:RU * SUB + T_SH] = \
            w_bias_f[N_USERS + c * T_SH: N_USERS + (c + 1) * T_SH]
        wa[(RU + RT) * SUB:(RU + RT) * SUB + B_SH] = \
            w_bias_f[N_USERS + N_ITEMS + c * B_SH: N_USERS + N_ITEMS + (c + 1) * B_SH]
        m = {
            "xall": xa,
            "wall": wa,
            "uV": u_V[c * U_SH:(c + 1) * U_SH],
            "tV": t_V[c * T_SH:(c + 1) * T_SH],
            "bV": b_V[c * B_SH:(c + 1) * B_SH],
            "iota1": i1_np,
        }
        if COLLECTIVE:
            m["coef5"] = coef5
            m["w0d"] = w0
            m["delta"] = delta_f
        in_maps.append(m)

    nc = _get_nc(COLLECTIVE)
    trace = os.environ.get("BFM_TRACE", "0") == "1"
    kwargs = {}
    if trace:
        kwargs["trace"] = True
    res = run_bass_kernel_spmd(nc, in_maps, core_ids=list(range(NCORES)), **kwargs)
    kernel._last_results = res

    if COLLECTIVE:
        return np.asarray(res.results[0]["out"], dtype=np.float32).reshape(1, 1)

    # host epilogue: unshard (sum partial outputs) + scalar FM combination
    P = np.zeros((64, 6), dtype=np.float64)
    for c in range(NCORES):
        P += np.asarray(res.results[c]["part"], dtype=np.float64)
    u, t, sb, q = P[:, 0], P[:, 1], P[:, 2], P[:, 3]
    bias = P[0, 4]
    y = (float(w0[0]) + bias + GAMMA[0] * (u @ t) + GAMMA[1] * (t @ sb)
         + GAMMA[2] * 0.5 * ((sb @ sb) - q.sum()) + GAMMA[3] * (u @ sb))
    z = y * float(delta_f[0])
    out = 1.0 / (1.0 + np.exp(-z))
    return np.array([[out]], dtype=np.float32)


# revision 13
# speedup vs baseline: 1.2225x; 1.0362x over previous
"""Sparse factorization-machine forward on 8 Trainium2 NeuronCores.

Model: y = sigmoid(delta * (w0 + <x, w_bias> + u.t + t.Sb + 0.5(|Sb|^2 - sum_b||b_i||^2) + u.Sb))
where x is a sparse one/multi-hot transaction over [user(1M) | target(500K) | basket(500K)],
u = x_u @ u_V, t = x_t @ t_V, Sb = x_b @ b_V.

Strategy (embedding/model parallel, row sharding):
 - each core owns 1/8 of the rows of u_V/t_V/b_V/w_bias and the matching x slices
 - on device: scan the x slices for nonzeros (masked-iota keys + per-partition top-8),
   build int16 gather lists, dma_gather only the needed embedding rows from HBM,
   weighted-sum them, and reduce partials across partitions with PE matmuls
 - bias term <x, w_bias> is computed densely on-device (it is tiny)
 - partials (u/t/Sb/q vectors + bias scalar) are all-reduced across the 8 cores and
   the scalar FM combination + sigmoid computed on-device (collective mode), or
   returned per-core and combined on the host (a pure unshard-reduce + epilogue)
"""
import os
import sys

sys.path.insert(0, '/opt/trn_rl_repo')

import numpy as np

N_USERS = 1_000_000
N_ITEMS = 500_000
K = 64
NCORES = 8
GAMMA = (1.0, 1.0, 1.0, 1.0)

F = 256                 # free elems per partition per sub-range
SUB = 128 * F           # 32768 rows per sub-range (int16 gather index limit)
U_SH = N_USERS // NCORES     # 125000 rows/core
T_SH = N_ITEMS // NCORES     # 62500
B_SH = N_ITEMS // NCORES     # 62500
RU, RT, RB = 4, 2, 2    # sub-ranges per core per table
K_UT = 1                # gather slots per partition (one-hot ranges)
K_B = 3                 # gather slots per partition (basket range)

COLLECTIVE = os.environ.get("BFM_COLLECTIVE", "1") == "1"

_cache = {}


def _split_multi_waits(nc):
    """This walrus build accepts at most one sem-wait per instruction; hoist
    extras into standalone event-semaphore waits on the same engine."""
    from concourse import mybir
    n = 0
    for bb in nc.main_func.blocks:
        insts = bb.instructions
        out = []
        for ins in insts:
            si = ins.sync_info
            waits = list(si.on_wait) if si and si.on_wait else []
            if len(waits) > 1:
                for w in waits[1:]:
                    ev = mybir.InstEventSemaphore(name=f"{ins.name}-hw{n}", ins=[], outs=[])
                    ev.engine = ins.engine
                    ev.sync_info = mybir.SyncInfo(on_wait=[w], on_update=[])
                    out.append(ev)
                    n += 1
                si.on_wait = waits[:1]
            out.append(ins)
        if len(out) != len(insts):
            insts[:] = out
    return n


def _build(collective: bool):
    from concourse import bacc, mybir, tile, library_config

    fp32 = mybir.dt.float32
    i16 = mybir.dt.int16
    Alu = mybir.AluOpType

    nc = bacc.Bacc("TRN2", num_devices=NCORES, num_swdge_queues=4)

    XAW = 8 * F            # 2048 cols: u blocks 0-3, t 4-5, b 6-7
    NS = RU * K_UT + RT * K_UT + RB * K_B   # 12 gather slots

    xall_d = nc.dram_tensor("xall", [8 * SUB], fp32, kind="ExternalInput")
    wall_d = nc.dram_tensor("wall", [8 * SUB], fp32, kind="ExternalInput")
    uV_d = nc.dram_tensor("uV", [U_SH, K], fp32, kind="ExternalInput")
    tV_d = nc.dram_tensor("tV", [T_SH, K], fp32, kind="ExternalInput")
    bV_d = nc.dram_tensor("bV", [B_SH, K], fp32, kind="ExternalInput")
    i1_d = nc.dram_tensor("iota1", [128, XAW], fp32, kind="ExternalInput")
    part_d = nc.dram_tensor("part", [64, 6], fp32, kind="ExternalOutput")
    if collective:
        co_d = nc.dram_tensor("coef5", [8, 1], fp32, kind="ExternalInput")
        w0_d = nc.dram_tensor("w0d", [1], fp32, kind="ExternalInput")
        de_d = nc.dram_tensor("delta", [1], fp32, kind="ExternalInput")
        out_d = nc.dram_tensor("out", [1, 1], fp32, kind="ExternalOutput")

    # (tag, num sub-ranges, shard rows, table, slots per partition, slot offset, block offset)
    ranges = [
        ("u", RU, U_SH, uV_d, K_UT, 0, 0),
        ("t", RT, T_SH, tV_d, K_UT, RU * K_UT, RU),
        ("b", RB, B_SH, bV_d, K_B, RU * K_UT + RT * K_UT, RU + RT),
    ]

    with tile.TileContext(nc) as tc:
        with (
            tc.tile_pool(name="sbuf", bufs=1) as pool,
            tc.tile_pool(name="psum", bufs=1, space="PSUM") as psum,
            tc.tile_pool(name="dram", bufs=1, space="DRAM") as dram,
        ):
            nc.gpsimd.load_library(library_config.mlp)

            XA = pool.tile([128, XAW], fp32)
            IA = pool.tile([128, XAW], fp32)
            nc.sync.dma_start(
                XA[:], xall_d[:].rearrange("(p w) -> p w", p=128))
            nc.sync.dma_start(IA[:], i1_d[:])

            # ---- scan: keys = x * (local_idx+1), top-8 per partition ----
            KEYA = pool.tile([128, XAW], fp32)
            nc.vector.tensor_tensor(KEYA[:], XA[:], IA[:], Alu.mult)
            T8S = pool.tile([128, 64], fp32)
            for r in range(8):
                nc.vector.max(T8S[:, 8 * r:8 * r + 8], KEYA[:, r * F:(r + 1) * F])

            # weights / int16 indices for the 12 gather slots
            W8S = pool.tile([128, NS], fp32)
            IDXALL = pool.tile([128, NS], i16)
            ut_keys = T8S[:, 0:48].rearrange("p (r e) -> p r e", e=8)[:, :, 0:K_UT]
            b_keys = T8S[:, 48:64].rearrange("p (r e) -> p r e", e=8)[:, :, 0:K_B]
            nc.vector.tensor_scalar(
                W8S[:, 0:6].rearrange("p (r e) -> p r e", e=K_UT),
                ut_keys, 1.0, None, Alu.is_ge)
            nc.vector.tensor_scalar(
                W8S[:, 6:NS].rearrange("p (r e) -> p r e", e=K_B),
                b_keys, 1.0, None, Alu.is_ge)
            nc.vector.tensor_scalar(
                IDXALL[:, 0:6].rearrange("p (r e) -> p r e", e=K_UT),
                ut_keys, 1.0, 0.0, Alu.subtract, Alu.max)
            nc.vector.tensor_scalar(
                IDXALL[:, 6:NS].rearrange("p (r e) -> p r e", e=K_B),
                b_keys, 1.0, 0.0, Alu.subtract, Alu.max)

            # ---- fold idx lists into 16-partition-wrapped layout via DRAM ----
            IW = NS * 8                    # 96 idx cols per partition row
            scr = dram.tile([16 * IW], i16)
            scr_v = scr[:].rearrange("(q w) -> w q", q=16, w=IW)
            eng_cycle = [nc.sync, nc.scalar]
            for j in range(NS):
                eng_cycle[j % 2].dma_start(
                    scr_v[8 * j:8 * j + 8], IDXALL[:, j:j + 1])
            IDXW = pool.tile([128, IW], i16)
            for g in range(8):
                eng_cycle[g % 2].dma_start(
                    IDXW[16 * g:16 * (g + 1), :],
                    scr[:].rearrange("(q w) -> q w", w=IW))

            # ---- gather embedding rows (SWDGE, spread across queues) ----
            GA = pool.tile([128, NS, K], fp32)
            qn = 0
            for tag, R, SH, tbl, k, soff, _blk in ranges:
                for r in range(R):
                    row0 = r * SUB
                    nrows = min(SUB, SH - row0)
                    for sslot in range(k):
                        j = soff + r * k + sslot
                        nc.gpsimd.dma_gather(
                            out_ap=GA[:, j:j + 1, :],
                            in_ap=tbl[row0:row0 + nrows, :],
                            idxs_ap=IDXW[:, 8 * j:8 * j + 8],
                            num_idxs=128, num_idxs_reg=128, elem_size=K,
                            queue_num=qn % 4,
                        )
                        qn += 1

            # ---- weighted sums ----
            WM = pool.tile([128, NS, K], fp32)
            wb_ap = W8S[:].rearrange("p (s one) -> p s one", one=1)
            wb_ap.ap[-1] = [0, K]
            nc.vector.tensor_tensor(WM[:], GA[:], wb_ap, Alu.mult)
            ACC = {}
            for tag, cols in (("u", (0, RU)), ("t", (RU, RU + RT)),
                              ("b", (6, NS))):
                a = pool.tile([128, K], fp32, tag=f"ACC{tag}")
                lo, hi = cols
                nc.vector.tensor_reduce(
                    a[:],
                    WM[:, lo:hi, :].rearrange("p s d -> p d s"),
                    axis=mybir.AxisListType.X, op=Alu.add)
                ACC[tag] = a

            # q = sum w * row^2 over basket slots
            nbs = RB * K_B
            SQ = pool.tile([128, nbs, K], fp32)
            nc.vector.tensor_tensor(SQ[:], GA[:, 6:NS, :], GA[:, 6:NS, :], Alu.mult)
            SQW = pool.tile([128, nbs, K], fp32)
            wqb_ap = W8S[:, 6:NS].rearrange("p (s one) -> p s one", one=1)
            wqb_ap.ap[-1] = [0, K]
            nc.vector.tensor_tensor(SQW[:], SQ[:], wqb_ap, Alu.mult)
            ACCQ = pool.tile([128, K], fp32)
            nc.vector.tensor_reduce(
                ACCQ[:], SQW[:].rearrange("p s d -> p d s"),
                axis=mybir.AxisListType.X, op=Alu.add)

            # ---- bias = <x, w_bias> ----
            WA = pool.tile([128, XAW], fp32)
            wa_dma = nc.scalar.dma_start(
                WA[:], wall_d[:].rearrange("(p w) -> p w", p=128))
            wa_dma.ins.bass_priority = 90000
            BP = pool.tile([128, XAW], fp32)
            bp_i = nc.vector.tensor_tensor(BP[:], XA[:], WA[:], Alu.mult)
            bp_i.ins.bass_priority = 90001
            bias_acc = pool.tile([128, 1], fp32)
            br_i = nc.vector.tensor_reduce(
                bias_acc[:], BP[:], axis=mybir.AxisListType.X, op=Alu.add)
            br_i.ins.bass_priority = 90002

            # ---- cross-partition reduce via PE (data stationary, ones moving) ----
            ones = pool.tile([128, 1], fp32)
            nc.vector.memset(ones[:], 1.0)
            PART = pool.tile([64, 6], fp32)
            nc.vector.memset(PART[:], 0.0)
            for i, tag in enumerate(("u", "t", "b")):
                pp = psum.tile([K, 1], fp32, tag=f"P{tag}")
                nc.tensor.matmul(pp[:], ACC[tag][:], ones[:], start=True, stop=True)
                nc.vector.tensor_copy(PART[:, i:i + 1], pp[:])
            ppq = psum.tile([K, 1], fp32, tag="Pq")
            nc.tensor.matmul(ppq[:], ACCQ[:], ones[:], start=True, stop=True)
            nc.vector.tensor_copy(PART[:, 3:4], ppq[:])
            ppb = psum.tile([1, 1], fp32, tag="Pbias")
            nc.tensor.matmul(ppb[:], bias_acc[:], ones[:], start=True, stop=True)
            nc.vector.tensor_copy(PART[0:1, 4:5], ppb[:])

            nc.sync.dma_start(part_d[:], PART[:])

            if collective:
                in_b = dram.tile([64 * 6], fp32)
                out_b = dram.tile([64 * 6], fp32)
                nc.sync.dma_start(in_b[:].rearrange("(p c) -> p c", c=6), PART[:])
                nc.gpsimd.collective_compute(
                    "AllReduce", Alu.add,
                    replica_groups=[list(range(NCORES))],
                    ins=[in_b.opt()], outs=[out_b.opt()],
                )
                RED = pool.tile([64, 6], fp32)
                nc.sync.dma_start(RED[:], out_b[:].rearrange("(p c) -> p c", c=6))
                D = pool.tile([64, 5], fp32)
                nc.vector.tensor_tensor(D[:, 0:1], RED[:, 0:1], RED[:, 1:2], Alu.mult)
                nc.vector.tensor_tensor(D[:, 1:2], RED[:, 1:2], RED[:, 2:3], Alu.mult)
                nc.vector.tensor_tensor(D[:, 2:3], RED[:, 2:3], RED[:, 2:3], Alu.mult)
                nc.vector.tensor_tensor(D[:, 3:4], RED[:, 0:1], RED[:, 2:3], Alu.mult)
                nc.vector.tensor_copy(D[:, 4:5], RED[:, 3:4])
                PD = psum.tile([5, 1], fp32)
                nc.tensor.matmul(PD[:], D[:], ones[:64, :], start=True, stop=True)
                CO = pool.tile([8, 1], fp32)
                nc.sync.dma_start(CO[:], co_d[:])
                FD = pool.tile([5, 1], fp32)
                nc.vector.tensor_tensor(FD[:], PD[:], CO[:5, :], Alu.mult)
                PY = psum.tile([1, 1], fp32)
                nc.tensor.matmul(PY[:], FD[:], ones[:5, :], start=True, stop=True)
                W0T = pool.tile([1, 2], fp32)
                nc.sync.dma_start(W0T[0:1, 0:1], w0_d[:].rearrange("(a b) -> a b", a=1))
                nc.sync.dma_start(W0T[0:1, 1:2], de_d[:].rearrange("(a b) -> a b", a=1))
                Y = pool.tile([1, 3], fp32)
                nc.vector.tensor_tensor(Y[:, 0:1], PY[:], RED[0:1, 4:5], Alu.add)
                nc.vector.tensor_tensor(Y[:, 1:2], Y[:, 0:1], W0T[:, 0:1], Alu.add)
                nc.vector.tensor_tensor(Y[:, 2:3], Y[:, 1:2], W0T[:, 1:2], Alu.mult)
                OUT = pool.tile([1, 1], fp32)
                nc.scalar.activation(
                    OUT[:], Y[:, 2:3], mybir.ActivationFunctionType.Sigmoid)
                nc.sync.dma_start(out_d[:], OUT[:])

    nc.compile()
    return nc


def _get_nc(collective: bool):
    key = ("nc", collective)
    if key not in _cache:
        nc = _build(collective)
        _split_multi_waits(nc)
        _cache[key] = nc
    return _cache[key]


def kernel(x, delta, pmi, w_0, w_bias, u_V, t_V, b_V):
    from concourse.bass_utils import run_bass_kernel_spmd

    x = np.asarray(x, dtype=np.float32)
    w_bias_f = np.asarray(w_bias, dtype=np.float32).reshape(-1)
    u_V = np.ascontiguousarray(np.asarray(u_V, dtype=np.float32))
    t_V = np.ascontiguousarray(np.asarray(t_V, dtype=np.float32))
    b_V = np.ascontiguousarray(np.asarray(b_V, dtype=np.float32))
    w0 = np.asarray(w_0, dtype=np.float32).reshape(-1)
    delta_f = np.asarray(delta, dtype=np.float32).reshape(-1)

    PADR = 8 * SUB
    xall = np.zeros(PADR, dtype=np.float32)
    wall = np.zeros(PADR, dtype=np.float32)
    i1_np = np.tile(np.arange(SUB, dtype=np.float32).reshape(128, F) + 1.0, (1, 8))
    coef5 = np.array([[GAMMA[0]], [GAMMA[1]], [0.5 * GAMMA[2]], [GAMMA[3]],
                      [-0.5 * GAMMA[2]], [0.0], [0.0], [0.0]], dtype=np.float32)

    in_maps = []
    for c in range(NCORES):
        xa = np.zeros(PADR, dtype=np.float32)
        wa = np.zeros(PADR, dtype=np.float32)
        xa[0:U_SH] = x[c * U_SH:(c + 1) * U_SH]
        xa[RU * SUB:RU * SUB + T_SH] = x[N_USERS + c * T_SH: N_USERS + (c + 1) * T_SH]
        xa[(RU + RT) * SUB:(RU + RT) * SUB + B_SH] = \
            x[N_USERS + N_ITEMS + c * B_SH: N_USERS + N_ITEMS + (c + 1) * B_SH]
        wa[0:U_SH] = w_bias_f[c * U_SH:(c + 1) * U_SH]
        wa[RU * SUB:RU * SUB + T_SH] = \
            w_bias_f[N_USERS + c * T_SH: N_USERS + (c + 1) * T_SH]
        wa[(RU + RT) * SUB:(RU + RT) * SUB + B_SH] = \
            w_bias_f[N_USERS + N_ITEMS + c * B_SH: N_USERS + N_ITEMS + (c + 1) * B_SH]
        xa = np.ascontiguousarray(
            xa.reshape(8, 128, F).transpose(1, 0, 2)).reshape(-1)
        wa = np.ascontiguousarray(
            wa.reshape(8, 128, F).transpose(1, 0, 2)).reshape(-1)
        m = {
            "xall": xa,
            "wall": wa,
            "uV": u_V[c * U_SH:(c + 1) * U_SH],
            "tV": t_V[c * T_SH:(c + 1) * T_SH],
            "bV": b_V[c * B_SH:(c + 1) * B_SH],
            "iota1": i1_np,
        }
        if COLLECTIVE:
            m["coef5"] = coef5
            m["w0d"] = w0
            m["delta"] = delta_f
        in_maps.append(m)

    nc = _get_nc(COLLECTIVE)
    trace = os.environ.get("BFM_TRACE", "0") == "1"
    kwargs = {}
    if trace:
        kwargs["trace"] = True
    res = run_bass_kernel_spmd(nc, in_maps, core_ids=list(range(NCORES)), **kwargs)
    kernel._last_results = res

    if COLLECTIVE:
        return np.asarray(res.results[0]["out"], dtype=np.float32).reshape(1, 1)

    # host epilogue: unshard (sum partial outputs) + scalar FM combination
    P = np.zeros((64, 6), dtype=np.float64)
    for c in range(NCORES):
        P += np.asarray(res.results[c]["part"], dtype=np.float64)
    u, t, sb, q = P[:, 0], P[:, 1], P[:, 2], P[:, 3]
    bias = P[0, 4]
    y = (float(w0[0]) + bias + GAMMA[0] * (u @ t) + GAMMA[1] * (t @ sb)
         + GAMMA[2] * 0.5 * ((sb @ sb) - q.sum()) + GAMMA[3] * (u @ sb))
    z = y * float(delta_f[0])
    out = 1.0 / (1.0 + np.exp(-z))
    return np.array([[out]], dtype=np.float32)
